# revision 11
# baseline (speedup 1.0000x reference)
"""Trainium2 Bass kernel for nn_CRAMForCausalLM.

Sharding: 8-way data-parallel over tokens (each core owns 256 contiguous
tokens of one batch element, plus a 32-token halo so the EMA retention scan
is computed locally — contributions older than 32 steps are damped by 0.5^32
< 3e-10, below f32 noise).  The LM head is vocab-sharded (each core computes
4000 logit rows for all 2048 tokens) fed by a single AllGather of the final
hidden states.  Activations live feature-major ([H, tokens]) on chip so every
GEMM chains without transposes; LayerNorm stats use ones-matmuls on the
TensorEngine; the EMA recurrence uses the VectorEngine tensor_tensor_scan.
"""

import numpy as np

import concourse.bass as bass
import concourse.bacc as bacc
import concourse.tile as tile
import concourse.mybir as mybir
import concourse.bass_utils as bass_utils
import os as _os

LAST_EXEC_NS = None


def _maybe_install_trace_hook():
    import contextlib, ctypes, sys, types
    if "antenv.axon_hooks" in sys.modules:
        return
    lib = ctypes.CDLL("/opt/axon/libaxon_pjrt.so")
    if not hasattr(lib, "axon_start_nrt_profile"):
        return
    lib.axon_start_nrt_profile.argtypes = [ctypes.POINTER(ctypes.c_int64), ctypes.c_size_t]
    lib.axon_start_nrt_profile.restype = ctypes.c_int64
    lib.axon_stop_nrt_profile.argtypes = [ctypes.c_char_p]
    lib.axon_stop_nrt_profile.restype = ctypes.c_int64

    @contextlib.contextmanager
    def _hook(output_dir, device_ids):
        import jax
        jax.devices()
        if device_ids:
            ids = (ctypes.c_int64 * len(device_ids))(*device_ids)
            rc = lib.axon_start_nrt_profile(ids, len(device_ids))
        else:
            rc = lib.axon_start_nrt_profile(None, 0)
        if rc != 0:
            raise RuntimeError(f"axon_start_nrt_profile rc={rc}")
        try:
            yield
        finally:
            lib.axon_stop_nrt_profile(str(output_dir).encode())

    mod = types.ModuleType("antenv.axon_hooks")
    mod.get_axon_ntff_profile_hook = lambda: _hook
    mod.set_axon_ntff_profile_hook = lambda h: None
    sys.modules["antenv.axon_hooks"] = mod

AF = mybir.ActivationFunctionType
OP = mybir.AluOpType

B, S, H, F, L, V = 2, 1024, 1024, 4096, 8, 32000
EPS = 1e-5
NCORES = 8
HALO = 32
TM = 256            # main tokens per core
T = TM + HALO       # 288 tokens processed per core
TPAD = 384          # padded to 3 x 128 for the embedding gather
KH = H // 128       # 8 k-chunks over H
MH = H // 128       # 8 m-tiles over H
MF = F // 128       # 32 m-tiles over F
VS = V // NCORES    # 4000 vocab rows per core
VSP = 4096          # padded vocab rows per core
TALL = B * S        # 2048 total tokens

f32 = mybir.dt.float32
f32r = mybir.dt.float32r
bf16 = mybir.dt.bfloat16
i32 = mybir.dt.int32

_compiled = {}


def _swz(w, kp=128, mf=128):
    """[K, M] -> [mt, kp, kc*mf] so lhsT tile (mt, kc) = sbuf[:, kc*mf:(kc+1)*mf]."""
    K, M = w.shape
    kc, mt = K // kp, M // mf
    return np.ascontiguousarray(
        w.reshape(kc, kp, mt, mf).transpose(2, 1, 0, 3).reshape(mt, kp, kc * mf)
    )


def _cols(v, mt, width=128):
    """[M] -> [width, mt] so column j is v[j*width:(j+1)*width]."""
    return np.ascontiguousarray(v.reshape(mt, width).T)


def _build(gemm_bf16, ln_scaled):
    """Build + compile the Bass program.

    gemm_bf16: all large GEMMs (ret/ffn/lm + LN stats) in bf16; the
               residual stream stays f32r.  Else float32r everywhere.
    ln_scaled: apply LayerNorm scale/bias tensors (else they are known
               to be identity and are skipped)
    """
    nc = bacc.Bacc("TRN2", target_bir_lowering=False, debug=False,
                   num_devices=NCORES)
    wdt = bf16 if gemm_bf16 else f32r

    # ---- DRAM I/O ----
    ids_d = nc.dram_tensor("ids", [3, 128], i32, kind="ExternalInput")
    pos_d = nc.dram_tensor("pos", [3, 128, H], f32, kind="ExternalInput")
    wemb_d = nc.dram_tensor("wemb", [V, H], f32, kind="ExternalInput")
    retw_d = nc.dram_tensor("retw", [L, MH, 128, KH * 128], wdt, kind="ExternalInput")
    retb_d = nc.dram_tensor("retb", [L, 128, MH], f32, kind="ExternalInput")
    w1_d = nc.dram_tensor("w1", [L, MF, 128, KH * 128], wdt, kind="ExternalInput")
    b1_d = nc.dram_tensor("b1", [L, 128, MF], f32, kind="ExternalInput")
    w2_d = nc.dram_tensor("w2", [L, MH, 128, MF * 128], wdt, kind="ExternalInput")
    b2_d = nc.dram_tensor("b2", [L, 128, MH], f32, kind="ExternalInput")
    lmw_d = nc.dram_tensor("lmw", [VSP // 128, 128, KH * 128], wdt, kind="ExternalInput")
    mask_d = nc.dram_tensor("mask", [128, 1], f32, kind="ExternalInput")
    csret_d = nc.dram_tensor("csret", [L, 1, H], wdt, kind="ExternalInput")
    csw1_d = nc.dram_tensor("csw1", [L, 1, F], wdt, kind="ExternalInput")
    if ln_scaled:
        # 2 (s, b) x (emb, per-layer ln1, per-layer ln2, fin): [128, MH] each
        lns_d = nc.dram_tensor("lns", [2 * L + 2, 2, 128, MH], f32, kind="ExternalInput")
    out_d = nc.dram_tensor("logits", [VSP, TALL], f32, kind="ExternalOutput")

    with tile.TileContext(nc) as tc:
        with tc.tile_pool(name="per", bufs=1) as per, \
             tc.tile_pool(name="gpool", bufs=1) as gpool, \
             tc.tile_pool(name="lnout", bufs=2) as lnout:
            # persistent tiles
            xt = [per.tile([128, T], f32r, tag=f"xt{k}", name=f"xt{k}") for k in range(KH)]
            y1 = [per.tile([128, T], f32r, tag=f"y1{k}", name=f"y1{k}") for k in range(KH)]
            hres = [per.tile([128, T], f32r, tag=f"h{k}", name=f"h{k}") for k in range(KH)]
            yb1 = [per.tile([128, T], wdt, tag=f"yb1{k}", name=f"yb1{k}") for k in range(KH)]
            yb2 = [per.tile([128, T], wdt, tag=f"yb2{k}", name=f"yb2{k}") for k in range(KH)]
            g = [gpool.tile([128, T], wdt, tag=f"g{k}", name=f"g{k}") for k in range(MF)]
            half = per.tile([128, T], f32)
            nc.gpsimd.memset(half[:], 0.5)
            ones_f = per.tile([128, 1], f32)
            nc.gpsimd.memset(ones_f[:], 1.0)
            ones = per.tile([128, 1], wdt)
            nc.vector.tensor_copy(ones[:], ones_f[:])
            onesr_f = per.tile([1, 128], f32)
            nc.gpsimd.memset(onesr_f[:], 1.0)
            onesr = per.tile([1, 128], f32r)
            nc.vector.tensor_copy(onesr[:], onesr_f[:])
            mask = per.tile([128, 1], f32)
            nc.sync.dma_start(mask[:], mask_d.ap())
            epsc = per.tile([128, 1], f32)
            nc.gpsimd.memset(epsc[:], EPS)
            ident = per.tile([128, 128], f32)
            from concourse.masks import make_identity
            make_identity(nc, ident[:])
            if ln_scaled:
                lnt = per.tile([128, (2 * L + 2) * 2 * MH], f32)
                nc.sync.dma_start(
                    lnt[:],
                    lns_d.ap().rearrange("a b p m -> p (a b m)"))
            else:
                lnt = None

            def ln_cols(slot):
                if lnt is None:
                    return None, None
                off = slot * 2 * MH
                return lnt[:, off:off + MH], lnt[:, off + MH:off + 2 * MH]

            # ---------- LN stats (feature-major, off PE critical path) ----
            def ln_stats(ps_stat, ps_bc, tmp, yin, ybf):
                """Casts yin into ybf (GEMM dtype) and computes per-token
                stats.  Returns dict with nm (f32r [1,T]), nm_g (gemm-dtype),
                r_b / nmb_sb ([128,T] f32 SBUF)."""
                for k in range(KH):
                    nc.vector.tensor_copy(ybf[k][:], yin[k][:].bitcast(f32))
                sq = []
                for k in range(KH):
                    s = tmp.tile([128, T], wdt, tag="sq", name="sq")
                    nc.vector.tensor_tensor(s[:], ybf[k][:], ybf[k][:], OP.mult)
                    sq.append(s)
                p_st = ps_stat.tile([33, T], f32, tag="pst")
                p_sy = p_st[0:1, :]
                p_sq = p_st[32:33, :]
                for k in range(KH):
                    nc.tensor.matmul(p_sy, ones[:], ybf[k][:],
                                     start=(k == 0), stop=(k == KH - 1))
                for k in range(KH):
                    nc.tensor.matmul(p_sq, ones[:], sq[k][:],
                                     start=(k == 0), stop=(k == KH - 1))
                nm = lnout.tile([1, T], f32r, tag="nm", name="nm")
                nc.vector.tensor_scalar_mul(nm[:], p_sy, -1.0 / H)
                nm_g = lnout.tile([1, T], wdt, tag="nmg", name="nmg")
                nc.vector.tensor_copy(nm_g[:], nm[:].bitcast(f32))
                v1 = tmp.tile([1, T], f32, tag="v1")
                nc.vector.tensor_scalar_mul(v1[:], p_sq, 1.0 / H)
                m2 = tmp.tile([1, T], f32, tag="m2")
                nc.vector.tensor_tensor(m2[:], nm[:].bitcast(f32),
                                        nm[:].bitcast(f32), OP.mult)
                var = tmp.tile([1, T], f32r, tag="var")
                nc.vector.tensor_tensor(var[:], v1[:], m2[:], OP.subtract)
                p_vb = ps_bc.tile([128, T], f32, tag="bc", name="p_vb")
                nc.tensor.matmul(p_vb[:], onesr[:], var[:], start=True, stop=True)
                r_b = lnout.tile([128, T], f32, tag="rb", name="r_b")
                # rsqrt(var + eps) in one LUT pass (verified 4e-5 max rel err)
                nc.scalar.activation(r_b[:], p_vb[:], AF.Abs_reciprocal_sqrt,
                                     bias=epsc[:])
                p_nmb = ps_bc.tile([128, T], f32, tag="bc", name="p_nmb")
                nc.tensor.matmul(p_nmb[:], onesr[:], nm[:], start=True, stop=True)
                nmb_sb = lnout.tile([128, T], f32, tag="nmsb", name="nmb_sb")
                nc.scalar.copy(nmb_sb[:], p_nmb[:])
                return {"nm": nm, "nm_g": nm_g, "r_b": r_b, "nmb_sb": nmb_sb}

            # ---------- LN apply (residual-stream normalize) ----------
            def ln_apply(tmp, yin, st, yout, slot):
                scol, bcol = ln_cols(slot)
                for k in range(KH):
                    z = tmp.tile([128, T], f32, tag="z", name="z")
                    nc.vector.tensor_tensor(z[:], yin[k][:].bitcast(f32),
                                            st["nmb_sb"][:], OP.add)
                    if scol is None:
                        nc.vector.tensor_tensor(yout[k][:], z[:],
                                                st["r_b"][:], OP.mult)
                    else:
                        z2 = tmp.tile([128, T], f32, tag="z2", name="z2")
                        nc.vector.tensor_tensor(z2[:], z[:], st["r_b"][:],
                                                OP.mult)
                        nc.vector.tensor_scalar(
                            yout[k][:], z2[:],
                            scol[:, k:k + 1], bcol[:, k:k + 1], OP.mult, OP.add)

            # ================= Embedding =================
            with tc.tile_pool(name="emb", bufs=2) as ep, \
                 tc.tile_pool(name="pse", bufs=3, space="PSUM") as pse:
                for c in range(3):
                    idx = ep.tile([128, 1], i32, tag="idx")
                    nc.sync.dma_start(idx[:], ids_d.ap()[c][:, None])
                    gt = ep.tile([128, H], f32, tag="gt")
                    nc.gpsimd.indirect_dma_start(
                        out=gt[:], out_offset=None, in_=wemb_d.ap(),
                        in_offset=bass.IndirectOffsetOnAxis(ap=idx[:, :1], axis=0))
                    pt = ep.tile([128, H], f32, tag="pt")
                    nc.sync.dma_start(pt[:], pos_d.ap()[c])
                    nc.vector.tensor_tensor(gt[:], gt[:], pt[:], OP.add)
                    cnt = T - 256 if c == 2 else 128
                    for k in range(KH):
                        ptr = pse.tile([128, 128], f32, tag="ptr")
                        nc.tensor.transpose(ptr[:], gt[:, k * 128:(k + 1) * 128],
                                            ident[:])
                        nc.vector.tensor_copy(
                            y1[k][:, c * 128:c * 128 + cnt], ptr[:, :cnt])

            # ================= Layers =================
            with tc.tile_pool(name="wret", bufs=3) as wret, \
                 tc.tile_pool(name="w1p", bufs=4) as w1p, \
                 tc.tile_pool(name="w2p", bufs=3) as w2p, \
                 tc.tile_pool(name="bias", bufs=2) as biasp, \
                 tc.tile_pool(name="tmp", bufs=3) as tmp, \
                 tc.tile_pool(name="psmm", bufs=4, space="PSUM") as psmm, \
                 tc.tile_pool(name="psst", bufs=1, space="PSUM") as ps_stat, \
                 tc.tile_pool(name="psbc", bufs=2, space="PSUM") as ps_bc:

                # embedding LN feeds the fused layer-0 retention GEMM
                st2 = ln_stats(ps_stat, ps_bc, tmp, y1, yb2)
                ln_apply(tmp, y1, st2, xt, 0)

                for l in range(L):
                    retb = biasp.tile([128, MH], f32, tag="retb")
                    nc.sync.dma_start(retb[:], retb_d.ap()[l])
                    b1 = biasp.tile([128, MF], f32, tag="b1")
                    nc.sync.dma_start(b1[:], b1_d.ap()[l])
                    b2 = biasp.tile([128, MH], f32, tag="b2")
                    nc.sync.dma_start(b2[:], b2_d.ap()[l])
                    csr = biasp.tile([1, H], wdt, tag="csr")
                    nc.sync.dma_start(csr[:], csret_d.ap()[l])
                    cs1 = biasp.tile([1, F], wdt, tag="cs1")
                    nc.sync.dma_start(cs1[:], csw1_d.ap()[l])

                    # --- retention GEMM (fused with preceding LN) ---
                    for mt in range(MH):
                        wt = wret.tile([128, KH * 128], wdt, tag="wret")
                        nc.sync.dma_start(wt[:], retw_d.ap()[l, mt])
                        ps = psmm.tile([128, T], f32, tag="mm")
                        for kc in range(KH):
                            nc.tensor.matmul(
                                ps[:], wt[:, kc * 128:(kc + 1) * 128], yb2[kc][:],
                                start=(kc == 0), stop=False)
                        nc.tensor.matmul(
                            ps[:], csr[:, mt * 128:(mt + 1) * 128], st2["nm_g"][:],
                            start=False, stop=True)
                        sgin = tmp.tile([128, T], f32, tag="sgin", name="sgin")
                        nc.vector.tensor_tensor(sgin[:], ps[:], st2["r_b"][:],
                                                OP.mult)
                        s = tmp.tile([128, T], f32, tag="sig", name="sig")
                        nc.scalar.activation(s[:], sgin[:], AF.Sigmoid,
                                             bias=retb[:, mt:mt + 1])
                        nc.vector.tensor_scalar_mul(
                            s[:, :HALO], s[:, :HALO], mask[:, :1])
                        stt = tmp.tile([128, T], f32, tag="scan", name="scan")
                        nc.vector.tensor_tensor_scan(
                            stt[:], half[:], s[:], 0.0, OP.mult, OP.add)
                        # y1 = x + 0.5*scan_state   (f32r rounded on write)
                        nc.vector.scalar_tensor_tensor(
                            y1[mt][:], stt[:], 0.5, xt[mt][:].bitcast(f32),
                            OP.mult, OP.add)

                    # --- LN1 stats (feeds fused FFN1) + residual apply ---
                    st1 = ln_stats(ps_stat, ps_bc, tmp, y1, yb1)
                    ln_apply(tmp, y1, st1, hres, 1 + 2 * l)

                    # --- FFN1 + gelu (fused with LN1) ---
                    for mt in range(MF):
                        wt = w1p.tile([128, KH * 128], wdt, tag="w1")
                        nc.sync.dma_start(wt[:], w1_d.ap()[l, mt])
                        ps = psmm.tile([128, T], f32, tag="mm")
                        for kc in range(KH):
                            nc.tensor.matmul(
                                ps[:], wt[:, kc * 128:(kc + 1) * 128], yb1[kc][:],
                                start=(kc == 0), stop=False)
                        nc.tensor.matmul(
                            ps[:], cs1[:, mt * 128:(mt + 1) * 128], st1["nm_g"][:],
                            start=False, stop=True)
                        gin = tmp.tile([128, T], f32, tag="gin", name="gin")
                        nc.vector.tensor_tensor(gin[:], ps[:], st1["r_b"][:],
                                                OP.mult)
                        nc.scalar.activation(g[mt][:], gin[:], AF.Gelu_apprx_tanh,
                                             bias=b1[:, mt:mt + 1])

                    # --- FFN2 ---
                    for mt in range(MH):
                        wt = w2p.tile([128, MF * 128], wdt, tag="w2")
                        nc.sync.dma_start(wt[:], w2_d.ap()[l, mt])
                        ps = psmm.tile([128, T], f32, tag="mm")
                        for kc in range(MF):
                            nc.tensor.matmul(
                                ps[:], wt[:, kc * 128:(kc + 1) * 128], g[kc][:],
                                start=(kc == 0), stop=(kc == MF - 1))
                        # y1 = (ffn + b2) + h    (becomes LN2 input)
                        nc.vector.scalar_tensor_tensor(
                            y1[mt][:], ps[:], b2[:, mt:mt + 1],
                            hres[mt][:].bitcast(f32), OP.add, OP.add)

                    # --- LN2 stats (feeds next layer's fused ret GEMM) ---
                    st2 = ln_stats(ps_stat, ps_bc, tmp, y1, yb2)
                    ln_apply(tmp, y1, st2, xt, 2 + 2 * l)

                # final LN: classic (stats + full apply) -> xf in GEMM dtype
                stf = ln_stats(ps_stat, ps_bc, tmp, xt, yb1)
                xf = yb2
                ln_apply(tmp, xt, stf, xf, 2 * L + 1)

            # ================= AllGather of final hidden =================
            with tc.tile_pool(name="dram", bufs=1, space="DRAM") as dramp:
                xdt = wdt
                bnc = dramp.tile([H, TM], xdt)
                for k in range(KH):
                    nc.sync.dma_start(bnc[k * 128:(k + 1) * 128, :],
                                      xf[k][:, HALO:T])
                xg = dramp.tile([NCORES, H, TM], xdt, addr_space="Shared")
                nc.gpsimd.collective_compute(
                    "AllGather", OP.bypass,
                    replica_groups=[list(range(NCORES))],
                    ins=[bnc.opt()], outs=[xg.opt()])

                # ================= LM head =================
                with tc.tile_pool(name="lmx", bufs=1) as lmx, \
                     tc.tile_pool(name="lmw", bufs=3) as lmwp, \
                     tc.tile_pool(name="lmo", bufs=4) as lmo, \
                     tc.tile_pool(name="pslm", bufs=6, space="PSUM") as pslm:
                    NRR = TALL // 512        # 4 psum column groups
                    rhs = [[None] * NRR for _ in range(KH)]
                    for kc in range(KH):
                        for rr in range(NRR):
                            t_ = lmx.tile([128, 512], xdt, tag=f"rhs{kc}_{rr}", name=f"rhs{kc}_{rr}")
                            nc.sync.dma_start(
                                t_[:, 0:TM],
                                xg[2 * rr, kc * 128:(kc + 1) * 128, :])
                            nc.sync.dma_start(
                                t_[:, TM:512],
                                xg[2 * rr + 1, kc * 128:(kc + 1) * 128, :])
                            rhs[kc][rr] = t_
                    for mt in range(VSP // 128):
                        wt = lmwp.tile([128, KH * 128], wdt, tag="lmw")
                        nc.sync.dma_start(wt[:], lmw_d.ap()[mt])
                        for rr in range(NRR):
                            ps = pslm.tile([128, 512], f32, tag="lm")
                            for kc in range(KH):
                                nc.tensor.matmul(
                                    ps[:], wt[:, kc * 128:(kc + 1) * 128],
                                    rhs[kc][rr][:],
                                    start=(kc == 0), stop=(kc == KH - 1))
                            ob = lmo.tile([128, 512], f32, tag="ob")
                            nc.any.tensor_copy(ob[:], ps[:])
                            nc.sync.dma_start(
                                out_d.ap()[mt * 128:(mt + 1) * 128,
                                           rr * 512:(rr + 1) * 512],
                                ob[:])

    nc.compile()
    return nc


def _prep_inputs(inputs, gemm_bf16, ln_scaled):
    import ml_dtypes
    wdtype = ml_dtypes.bfloat16 if gemm_bf16 else np.float32
    ids = np.asarray(inputs["input_ids"], np.int32)          # [B, S]
    retw_raw = [np.asarray(inputs["ret_W"][l], np.float32) for l in range(L)]
    w1_raw = [np.asarray(inputs["ffn_W1"][l], np.float32) for l in range(L)]
    retb_raw = [np.asarray(inputs["ret_b"][l], np.float32) for l in range(L)]
    b1_raw = [np.asarray(inputs["ffn_b1"][l], np.float32) for l in range(L)]
    if ln_scaled:
        # fold LN scale/bias of the LN feeding each fused GEMM into W / bias
        for l in range(L):
            s_in = (np.asarray(inputs["emb_ln_s"], np.float32) if l == 0
                    else np.asarray(inputs["ln2_s"][l - 1], np.float32))
            b_in = (np.asarray(inputs["emb_ln_b"], np.float32) if l == 0
                    else np.asarray(inputs["ln2_b"][l - 1], np.float32))
            retb_raw[l] = retb_raw[l] + b_in @ retw_raw[l]
            retw_raw[l] = retw_raw[l] * s_in[:, None]
            s1 = np.asarray(inputs["ln1_s"][l], np.float32)
            b1_ = np.asarray(inputs["ln1_b"][l], np.float32)
            b1_raw[l] = b1_raw[l] + b1_ @ w1_raw[l]
            w1_raw[l] = w1_raw[l] * s1[:, None]
    csret = np.stack([w.sum(0) for w in retw_raw]).reshape(L, 1, H).astype(wdtype)
    csw1 = np.stack([w.sum(0) for w in w1_raw]).reshape(L, 1, F).astype(wdtype)
    retw = np.stack([_swz(w) for w in retw_raw]).astype(wdtype)
    w1 = np.stack([_swz(w) for w in w1_raw]).astype(wdtype)
    w2 = np.stack([_swz(np.asarray(inputs["ffn_W2"][l], np.float32))
                   for l in range(L)]).astype(wdtype)
    retb = np.stack([_cols(v, MH) for v in retb_raw])
    b1 = np.stack([_cols(v, MF) for v in b1_raw])
    b2 = np.stack([_cols(np.asarray(inputs["ffn_b2"][l], np.float32), MH)
                   for l in range(L)])
    lmw_full = np.asarray(inputs["lm_W"], np.float32)         # [H, V]
    pos_emb = np.asarray(inputs["pos_emb"], np.float32)       # [S, H]
    wemb = np.ascontiguousarray(np.asarray(inputs["word_emb"], np.float32))

    common = {
        "wemb": wemb, "retw": retw, "retb": retb,
        "w1": w1, "b1": b1, "w2": w2, "b2": b2,
        "csret": csret, "csw1": csw1,
    }
    if ln_scaled:
        slots = [( np.asarray(inputs["emb_ln_s"], np.float32),
                   np.asarray(inputs["emb_ln_b"], np.float32))]
        for l in range(L):
            slots.append((np.asarray(inputs["ln1_s"][l], np.float32),
                          np.asarray(inputs["ln1_b"][l], np.float32)))
            slots.append((np.asarray(inputs["ln2_s"][l], np.float32),
                          np.asarray(inputs["ln2_b"][l], np.float32)))
        slots.append((np.asarray(inputs["fin_ln_s"], np.float32),
                      np.asarray(inputs["fin_ln_b"], np.float32)))
        lns = np.stack([np.stack([_cols(s, MH), _cols(b, MH)]) for s, b in slots])
        common["lns"] = lns

    in_maps = []
    for c in range(NCORES):
        b = c // (NCORES // B)
        s0 = TM * (c % (NCORES // B))
        if s0 == 0:
            hids = ids[b, 0:HALO]
            hpos = np.arange(HALO)
        else:
            hids = ids[b, s0 - HALO:s0]
            hpos = np.arange(s0 - HALO, s0)
        cids = np.concatenate([hids, ids[b, s0:s0 + TM],
                               np.zeros(TPAD - T, np.int32)]).astype(np.int32)
        cpos = np.concatenate([hpos, np.arange(s0, s0 + TM),
                               np.zeros(TPAD - T, np.int64)])
        pos = pos_emb[cpos].reshape(3, 128, H)
        lmw_c = np.zeros((H, VSP), np.float32)
        lmw_c[:, :VS] = lmw_full[:, c * VS:(c + 1) * VS]
        m = dict(common)
        m["mask"] = np.full((128, 1), 0.0 if s0 == 0 else 1.0, np.float32)
        m["ids"] = cids.reshape(3, 128)
        m["pos"] = np.ascontiguousarray(pos)
        m["lmw"] = _swz(lmw_c).astype(wdtype)
        in_maps.append(m)
    return in_maps


def kernel(**inputs):
    gemm_bf16 = _os.environ.get("KERNEL_GEMM_DT", "bf16") == "bf16"
    trivial = all(
        np.allclose(np.asarray(inputs[k]), 1.0) for k in
        ("emb_ln_s", "ln1_s", "ln2_s", "fin_ln_s")
    ) and all(
        np.allclose(np.asarray(inputs[k]), 0.0) for k in
        ("emb_ln_b", "ln1_b", "ln2_b", "fin_ln_b")
    )
    ln_scaled = not trivial

    key = (gemm_bf16, ln_scaled)
    if key not in _compiled:
        _compiled[key] = _build(gemm_bf16, ln_scaled)
    nc = _compiled[key]

    in_maps = _prep_inputs(inputs, gemm_bf16, ln_scaled)
    trace = bool(_os.environ.get("KERNEL_TRACE"))
    if trace:
        _maybe_install_trace_hook()
    res = bass_utils.run_bass_kernel_spmd(
        nc, in_maps, core_ids=list(range(NCORES)), trace=trace)
    global LAST_EXEC_NS
    LAST_EXEC_NS = res.exec_time_ns

    logits = np.empty((TALL, V), np.float32)
    for c in range(NCORES):
        logits[:, c * VS:(c + 1) * VS] = res.results[c]["logits"][:VS, :].T
    return logits.reshape(B, S, V)


# revision 12
# speedup vs baseline: 1.0486x; 1.0486x over previous
"""Trainium2 Bass kernel for nn_CRAMForCausalLM.

Sharding: 8-way data-parallel over tokens (each core owns 256 contiguous
tokens of one batch element, plus a 32-token halo so the EMA retention scan
is computed locally — contributions older than 32 steps are damped by 0.5^32
< 3e-10, below f32 noise).  The LM head is vocab-sharded (each core computes
4000 logit rows for all 2048 tokens) fed by a single AllGather of the final
hidden states.  Activations live feature-major ([H, tokens]) on chip so every
GEMM chains without transposes; LayerNorm stats use ones-matmuls on the
TensorEngine; the EMA recurrence uses the VectorEngine tensor_tensor_scan.
"""

import numpy as np

import concourse.bass as bass
import concourse.bacc as bacc
import concourse.tile as tile
import concourse.mybir as mybir
import concourse.bass_utils as bass_utils
import os as _os

LAST_EXEC_NS = None


def _maybe_install_trace_hook():
    import contextlib, ctypes, sys, types
    if "antenv.axon_hooks" in sys.modules:
        return
    lib = ctypes.CDLL("/opt/axon/libaxon_pjrt.so")
    if not hasattr(lib, "axon_start_nrt_profile"):
        return
    lib.axon_start_nrt_profile.argtypes = [ctypes.POINTER(ctypes.c_int64), ctypes.c_size_t]
    lib.axon_start_nrt_profile.restype = ctypes.c_int64
    lib.axon_stop_nrt_profile.argtypes = [ctypes.c_char_p]
    lib.axon_stop_nrt_profile.restype = ctypes.c_int64

    @contextlib.contextmanager
    def _hook(output_dir, device_ids):
        import jax
        jax.devices()
        if device_ids:
            ids = (ctypes.c_int64 * len(device_ids))(*device_ids)
            rc = lib.axon_start_nrt_profile(ids, len(device_ids))
        else:
            rc = lib.axon_start_nrt_profile(None, 0)
        if rc != 0:
            raise RuntimeError(f"axon_start_nrt_profile rc={rc}")
        try:
            yield
        finally:
            lib.axon_stop_nrt_profile(str(output_dir).encode())

    mod = types.ModuleType("antenv.axon_hooks")
    mod.get_axon_ntff_profile_hook = lambda: _hook
    mod.set_axon_ntff_profile_hook = lambda h: None
    sys.modules["antenv.axon_hooks"] = mod

AF = mybir.ActivationFunctionType
OP = mybir.AluOpType

B, S, H, F, L, V = 2, 1024, 1024, 4096, 8, 32000
EPS = 1e-5
NCORES = 8
HALO = 32
TM = 256            # main tokens per core
T = TM + HALO       # 288 tokens processed per core
TPAD = 384          # padded to 3 x 128 for the embedding gather
KH = H // 128       # 8 k-chunks over H
MH = H // 128       # 8 m-tiles over H
MF = F // 128       # 32 m-tiles over F
VS = V // NCORES    # 4000 vocab rows per core
VSP = 4096          # padded vocab rows per core
TALL = B * S        # 2048 total tokens

f32 = mybir.dt.float32
f32r = mybir.dt.float32r
bf16 = mybir.dt.bfloat16
i32 = mybir.dt.int32

_compiled = {}


def _swz(w, kp=128, mf=128):
    """[K, M] -> [mt, kp, kc*mf] so lhsT tile (mt, kc) = sbuf[:, kc*mf:(kc+1)*mf]."""
    K, M = w.shape
    kc, mt = K // kp, M // mf
    return np.ascontiguousarray(
        w.reshape(kc, kp, mt, mf).transpose(2, 1, 0, 3).reshape(mt, kp, kc * mf)
    )


def _cols(v, mt, width=128):
    """[M] -> [width, mt] so column j is v[j*width:(j+1)*width]."""
    return np.ascontiguousarray(v.reshape(mt, width).T)


def _build(gemm_bf16, ln_scaled):
    """Build + compile the Bass program.

    gemm_bf16: all large GEMMs (ret/ffn/lm + LN stats) in bf16; the
               residual stream stays f32r.  Else float32r everywhere.
    ln_scaled: apply LayerNorm scale/bias tensors (else they are known
               to be identity and are skipped)
    """
    nc = bacc.Bacc("TRN2", target_bir_lowering=False, debug=False,
                   num_devices=NCORES)
    wdt = bf16 if gemm_bf16 else f32r

    # ---- DRAM I/O ----
    ids_d = nc.dram_tensor("ids", [3, 128], i32, kind="ExternalInput")
    pos_d = nc.dram_tensor("pos", [3, 128, H], f32, kind="ExternalInput")
    wemb_d = nc.dram_tensor("wemb", [V, H], f32, kind="ExternalInput")
    retw_d = nc.dram_tensor("retw", [L, MH, 128, KH * 128], wdt, kind="ExternalInput")
    retb_d = nc.dram_tensor("retb", [L, 128, MH], f32, kind="ExternalInput")
    w1_d = nc.dram_tensor("w1", [L, MF, 128, KH * 128], wdt, kind="ExternalInput")
    b1_d = nc.dram_tensor("b1", [L, 128, MF], f32, kind="ExternalInput")
    w2_d = nc.dram_tensor("w2", [L, MH, 128, MF * 128], wdt, kind="ExternalInput")
    b2_d = nc.dram_tensor("b2", [L, 128, MH], f32, kind="ExternalInput")
    lmw_d = nc.dram_tensor("lmw", [VSP // 128, 128, KH * 128], wdt, kind="ExternalInput")
    mask_d = nc.dram_tensor("mask", [128, 1], f32, kind="ExternalInput")
    csret_d = nc.dram_tensor("csret", [L, 128, MH], f32, kind="ExternalInput")
    csw1_d = nc.dram_tensor("csw1", [L, 128, MF], f32, kind="ExternalInput")
    if ln_scaled:
        # 2 (s, b) x (emb, per-layer ln1, per-layer ln2, fin): [128, MH] each
        lns_d = nc.dram_tensor("lns", [2 * L + 2, 2, 128, MH], f32, kind="ExternalInput")
    out_d = nc.dram_tensor("logits", [VSP, TALL], f32, kind="ExternalOutput")

    with tile.TileContext(nc) as tc:
        with tc.tile_pool(name="per", bufs=1) as per, \
             tc.tile_pool(name="gpool", bufs=1) as gpool, \
             tc.tile_pool(name="lnout", bufs=2) as lnout:
            # persistent tiles
            xt = [per.tile([128, T], f32r, tag=f"xt{k}", name=f"xt{k}") for k in range(KH)]
            y1 = [per.tile([128, T], f32r, tag=f"y1{k}", name=f"y1{k}") for k in range(KH)]
            hres = [per.tile([128, T], f32r, tag=f"h{k}", name=f"h{k}") for k in range(KH)]
            yb1 = [per.tile([128, T], wdt, tag=f"yb1{k}", name=f"yb1{k}") for k in range(KH)]
            yb2 = [per.tile([128, T], wdt, tag=f"yb2{k}", name=f"yb2{k}") for k in range(KH)]
            sq1 = [per.tile([128, T], wdt, tag=f"sq1{k}", name=f"sq1{k}") for k in range(KH)]
            sq2 = [per.tile([128, T], wdt, tag=f"sq2{k}", name=f"sq2{k}") for k in range(KH)]
            g = [gpool.tile([128, T], wdt, tag=f"g{k}", name=f"g{k}") for k in range(MF)]
            half = per.tile([128, T], f32)
            nc.gpsimd.memset(half[:], 0.5)
            ones_f = per.tile([128, 1], f32)
            nc.gpsimd.memset(ones_f[:], 1.0)
            ones = per.tile([128, 1], wdt)
            nc.vector.tensor_copy(ones[:], ones_f[:])
            onesr_f = per.tile([1, 128], f32)
            nc.gpsimd.memset(onesr_f[:], 1.0)
            onesr = per.tile([1, 128], f32r)
            nc.vector.tensor_copy(onesr[:], onesr_f[:])
            mask = per.tile([128, 1], f32)
            nc.sync.dma_start(mask[:], mask_d.ap())
            epsc = per.tile([128, 1], f32)
            nc.gpsimd.memset(epsc[:], EPS)
            ident = per.tile([128, 128], f32)
            from concourse.masks import make_identity
            make_identity(nc, ident[:])
            if ln_scaled:
                lnt = per.tile([128, (2 * L + 2) * 2 * MH], f32)
                nc.sync.dma_start(
                    lnt[:],
                    lns_d.ap().rearrange("a b p m -> p (a b m)"))
            else:
                lnt = None

            def ln_cols(slot):
                if lnt is None:
                    return None, None
                off = slot * 2 * MH
                return lnt[:, off:off + MH], lnt[:, off + MH:off + 2 * MH]

            def cast_sq(k, src, ybf, sq):
                """bf16 cast + square of one chunk (inline after producer)."""
                nc.vector.tensor_copy(ybf[k][:], src[k][:].bitcast(f32))
                nc.vector.tensor_tensor(sq[k][:], ybf[k][:], ybf[k][:], OP.mult)

            # ---------- LN stats from pre-cast ybf/sq tiles ----------
            def ln_stats(ps_stat, ps_bc, tmp, ybf, sq):
                p_st = ps_stat.tile([33, T], f32, tag="pst")
                p_sy = p_st[0:1, :]
                p_sq = p_st[32:33, :]
                for k in range(KH):
                    nc.tensor.matmul(p_sy, ones[:], ybf[k][:],
                                     start=(k == 0), stop=(k == KH - 1))
                for k in range(KH):
                    nc.tensor.matmul(p_sq, ones[:], sq[k][:],
                                     start=(k == 0), stop=(k == KH - 1))
                nm = lnout.tile([1, T], f32r, tag="nm", name="nm")
                nc.vector.tensor_scalar_mul(nm[:], p_sy, -1.0 / H)
                v1 = tmp.tile([1, T], f32, tag="v1")
                nc.vector.tensor_scalar_mul(v1[:], p_sq, 1.0 / H)
                m2 = tmp.tile([1, T], f32, tag="m2")
                nc.vector.tensor_tensor(m2[:], nm[:].bitcast(f32),
                                        nm[:].bitcast(f32), OP.mult)
                var = tmp.tile([1, T], f32r, tag="var")
                nc.vector.tensor_tensor(var[:], v1[:], m2[:], OP.subtract)
                p_vb = ps_bc.tile([128, T], f32, tag="bc", name="p_vb")
                nc.tensor.matmul(p_vb[:], onesr[:], var[:], start=True, stop=True)
                r_b = lnout.tile([128, T], f32, tag="rb", name="r_b")
                nc.scalar.activation(r_b[:], p_vb[:], AF.Abs_reciprocal_sqrt,
                                     bias=epsc[:])
                p_nmb = ps_bc.tile([128, T], f32, tag="bc", name="p_nmb")
                nc.tensor.matmul(p_nmb[:], onesr[:], nm[:], start=True, stop=True)
                nmb_sb = lnout.tile([128, T], f32, tag="nmsb", name="nmb_sb")
                nc.scalar.copy(nmb_sb[:], p_nmb[:])
                # q = nm * r broadcast to [128, T]: rank-1 mean correction row
                q = tmp.tile([1, T], f32r, tag="q")
                nc.vector.tensor_tensor(q[:], nm[:].bitcast(f32), r_b[0:1, :],
                                        OP.mult)
                p_qb = ps_bc.tile([128, T], f32, tag="bc", name="p_qb")
                nc.tensor.matmul(p_qb[:], onesr[:], q[:], start=True, stop=True)
                q_sb = lnout.tile([128, T], f32, tag="qsb", name="q_sb")
                nc.scalar.copy(q_sb[:], p_qb[:])
                return {"r_b": r_b, "nmb_sb": nmb_sb, "q_sb": q_sb}

            # ---------- LN apply (residual-stream normalize) ----------
            def ln_apply(tmp, yin, st, yout, slot):
                scol, bcol = ln_cols(slot)
                for k in range(KH):
                    z = tmp.tile([128, T], f32, tag="z", name="z")
                    nc.vector.tensor_tensor(z[:], yin[k][:].bitcast(f32),
                                            st["nmb_sb"][:], OP.add)
                    if scol is None:
                        nc.vector.tensor_tensor(yout[k][:], z[:],
                                                st["r_b"][:], OP.mult)
                    else:
                        z2 = tmp.tile([128, T], f32, tag="z2", name="z2")
                        nc.vector.tensor_tensor(z2[:], z[:], st["r_b"][:],
                                                OP.mult)
                        nc.vector.tensor_scalar(
                            yout[k][:], z2[:],
                            scol[:, k:k + 1], bcol[:, k:k + 1], OP.mult, OP.add)

            def fused_epilogue(tmp, ps, st, cs_col, out, func, bias):
                """out = func(r*(ps + cs*nm) + bias) with cs_col [128,1]."""
                t = tmp.tile([128, T], f32, tag="ept", name="ept")
                nc.vector.tensor_tensor(t[:], ps[:], st["r_b"][:], OP.mult)
                fin = tmp.tile([128, T], f32, tag="epf", name="epf")
                nc.vector.scalar_tensor_tensor(
                    fin[:], st["q_sb"][:], cs_col, t[:], OP.mult, OP.add)
                nc.scalar.activation(out, fin[:], func, bias=bias)
                return fin

            # ================= Embedding =================
            with tc.tile_pool(name="emb", bufs=2) as ep, \
                 tc.tile_pool(name="pse", bufs=3, space="PSUM") as pse, \
                 tc.tile_pool(name="dramw", bufs=1, space="DRAM") as dramw:
                # tiny warm-up AllGather to absorb collective setup cost
                win = dramw.tile([128, 4], f32)
                nc.sync.dma_start(win[:], ident[:, :4])
                wout = dramw.tile([NCORES, 128, 4], f32, addr_space="Shared")
                nc.gpsimd.collective_compute(
                    "AllGather", OP.bypass,
                    replica_groups=[list(range(NCORES))],
                    ins=[win.opt()], outs=[wout.opt()])
                for c in range(3):
                    idx = ep.tile([128, 1], i32, tag="idx")
                    nc.sync.dma_start(idx[:], ids_d.ap()[c][:, None])
                    gt = ep.tile([128, H], f32, tag="gt")
                    nc.gpsimd.indirect_dma_start(
                        out=gt[:], out_offset=None, in_=wemb_d.ap(),
                        in_offset=bass.IndirectOffsetOnAxis(ap=idx[:, :1], axis=0))
                    pt = ep.tile([128, H], f32, tag="pt")
                    nc.sync.dma_start(pt[:], pos_d.ap()[c])
                    nc.vector.tensor_tensor(gt[:], gt[:], pt[:], OP.add)
                    cnt = T - 256 if c == 2 else 128
                    for k in range(KH):
                        ptr = pse.tile([128, 128], f32, tag="ptr")
                        nc.tensor.transpose(ptr[:], gt[:, k * 128:(k + 1) * 128],
                                            ident[:])
                        nc.vector.tensor_copy(
                            y1[k][:, c * 128:c * 128 + cnt], ptr[:, :cnt])
                for k in range(KH):
                    cast_sq(k, y1, yb2, sq2)

            # ================= Layers =================
            with tc.tile_pool(name="wret", bufs=3) as wret, \
                 tc.tile_pool(name="w1p", bufs=4) as w1p, \
                 tc.tile_pool(name="w2p", bufs=3) as w2p, \
                 tc.tile_pool(name="bias", bufs=2) as biasp, \
                 tc.tile_pool(name="tmp", bufs=3) as tmp, \
                 tc.tile_pool(name="psmm", bufs=4, space="PSUM") as psmm, \
                 tc.tile_pool(name="psst", bufs=1, space="PSUM") as ps_stat, \
                 tc.tile_pool(name="psbc", bufs=2, space="PSUM") as ps_bc:

                # embedding LN feeds the fused layer-0 retention GEMM
                st2 = ln_stats(ps_stat, ps_bc, tmp, yb2, sq2)
                ln_apply(tmp, y1, st2, xt, 0)

                for l in range(L):
                    retb = biasp.tile([128, MH], f32, tag="retb")
                    nc.sync.dma_start(retb[:], retb_d.ap()[l])
                    b1 = biasp.tile([128, MF], f32, tag="b1")
                    nc.sync.dma_start(b1[:], b1_d.ap()[l])
                    b2 = biasp.tile([128, MH], f32, tag="b2")
                    nc.sync.dma_start(b2[:], b2_d.ap()[l])
                    csr = biasp.tile([128, MH], f32, tag="csr")
                    nc.sync.dma_start(csr[:], csret_d.ap()[l])
                    cs1 = biasp.tile([128, MF], f32, tag="cs1")
                    nc.sync.dma_start(cs1[:], csw1_d.ap()[l])

                    # --- retention GEMM (fused with preceding LN) ---
                    for mt in range(MH):
                        wt = wret.tile([128, KH * 128], wdt, tag="wret")
                        nc.sync.dma_start(wt[:], retw_d.ap()[l, mt])
                        ps = psmm.tile([128, T], f32, tag="mm")
                        for kc in range(KH):
                            nc.tensor.matmul(
                                ps[:], wt[:, kc * 128:(kc + 1) * 128], yb2[kc][:],
                                start=(kc == 0), stop=(kc == KH - 1))
                        s = tmp.tile([128, T], f32, tag="sig", name="sig")
                        fused_epilogue(tmp, ps, st2, csr[:, mt:mt + 1],
                                       s[:], AF.Sigmoid, retb[:, mt:mt + 1])
                        nc.vector.tensor_scalar_mul(
                            s[:, :HALO], s[:, :HALO], mask[:, :1])
                        stt = tmp.tile([128, T], f32, tag="scan", name="scan")
                        nc.vector.tensor_tensor_scan(
                            stt[:], half[:], s[:], 0.0, OP.mult, OP.add)
                        # y1 = x + 0.5*scan_state   (f32r rounded on write)
                        nc.vector.scalar_tensor_tensor(
                            y1[mt][:], stt[:], 0.5, xt[mt][:].bitcast(f32),
                            OP.mult, OP.add)
                        cast_sq(mt, y1, yb1, sq1)

                    # --- LN1 stats (feeds fused FFN1) + residual apply ---
                    st1 = ln_stats(ps_stat, ps_bc, tmp, yb1, sq1)
                    ln_apply(tmp, y1, st1, hres, 1 + 2 * l)

                    # --- FFN1 + gelu (fused with LN1) ---
                    for mt in range(MF):
                        wt = w1p.tile([128, KH * 128], wdt, tag="w1")
                        nc.sync.dma_start(wt[:], w1_d.ap()[l, mt])
                        ps = psmm.tile([128, T], f32, tag="mm")
                        for kc in range(KH):
                            nc.tensor.matmul(
                                ps[:], wt[:, kc * 128:(kc + 1) * 128], yb1[kc][:],
                                start=(kc == 0), stop=(kc == KH - 1))
                        fused_epilogue(tmp, ps, st1, cs1[:, mt:mt + 1],
                                       g[mt][:], AF.Gelu_apprx_tanh,
                                       b1[:, mt:mt + 1])

                    # --- FFN2 ---
                    for mt in range(MH):
                        wt = w2p.tile([128, MF * 128], wdt, tag="w2")
                        nc.sync.dma_start(wt[:], w2_d.ap()[l, mt])
                        ps = psmm.tile([128, T], f32, tag="mm")
                        for kc in range(MF):
                            nc.tensor.matmul(
                                ps[:], wt[:, kc * 128:(kc + 1) * 128], g[kc][:],
                                start=(kc == 0), stop=(kc == MF - 1))
                        # y1 = (ffn + b2) + h    (becomes LN2 input)
                        nc.vector.scalar_tensor_tensor(
                            y1[mt][:], ps[:], b2[:, mt:mt + 1],
                            hres[mt][:].bitcast(f32), OP.add, OP.add)
                        cast_sq(mt, y1, yb2, sq2)

                    # --- LN2 stats (feeds next layer's fused ret GEMM) ---
                    st2 = ln_stats(ps_stat, ps_bc, tmp, yb2, sq2)
                    ln_apply(tmp, y1, st2, xt, 2 + 2 * l)

                # final LN: classic (stats + full apply) -> xf in GEMM dtype
                for k in range(KH):
                    cast_sq(k, xt, yb1, sq1)
                stf = ln_stats(ps_stat, ps_bc, tmp, yb1, sq1)
                xf = yb2
                ln_apply(tmp, xt, stf, xf, 2 * L + 1)

            # ================= AllGather of final hidden =================
            with tc.tile_pool(name="dram", bufs=1, space="DRAM") as dramp:
                xdt = wdt
                bnc = dramp.tile([H, TM], xdt)
                for k in range(KH):
                    nc.sync.dma_start(bnc[k * 128:(k + 1) * 128, :],
                                      xf[k][:, HALO:T])
                xg = dramp.tile([NCORES, H, TM], xdt, addr_space="Shared")
                nc.gpsimd.collective_compute(
                    "AllGather", OP.bypass,
                    replica_groups=[list(range(NCORES))],
                    ins=[bnc.opt()], outs=[xg.opt()])

                # ================= LM head =================
                with tc.tile_pool(name="lmx", bufs=1) as lmx, \
                     tc.tile_pool(name="lmw", bufs=6) as lmwp, \
                     tc.tile_pool(name="lmo", bufs=4) as lmo, \
                     tc.tile_pool(name="pslm", bufs=6, space="PSUM") as pslm:
                    NRR = TALL // 512        # 4 psum column groups
                    rhs = [[None] * NRR for _ in range(KH)]
                    for kc in range(KH):
                        for rr in range(NRR):
                            t_ = lmx.tile([128, 512], xdt, tag=f"rhs{kc}_{rr}", name=f"rhs{kc}_{rr}")
                            nc.sync.dma_start(
                                t_[:, 0:TM],
                                xg[2 * rr, kc * 128:(kc + 1) * 128, :])
                            nc.sync.dma_start(
                                t_[:, TM:512],
                                xg[2 * rr + 1, kc * 128:(kc + 1) * 128, :])
                            rhs[kc][rr] = t_
                    for mt in range(VSP // 128):
                        wt = lmwp.tile([128, KH * 128], wdt, tag="lmw")
                        nc.sync.dma_start(wt[:], lmw_d.ap()[mt])
                        for rr in range(NRR):
                            ps = pslm.tile([128, 512], f32, tag="lm")
                            for kc in range(KH):
                                nc.tensor.matmul(
                                    ps[:], wt[:, kc * 128:(kc + 1) * 128],
                                    rhs[kc][rr][:],
                                    start=(kc == 0), stop=(kc == KH - 1))
                            ob = lmo.tile([128, 512], f32, tag="ob")
                            nc.any.tensor_copy(ob[:], ps[:])
                            nc.sync.dma_start(
                                out_d.ap()[mt * 128:(mt + 1) * 128,
                                           rr * 512:(rr + 1) * 512],
                                ob[:])

    nc.compile()
    return nc


def _prep_inputs(inputs, gemm_bf16, ln_scaled):
    import ml_dtypes
    wdtype = ml_dtypes.bfloat16 if gemm_bf16 else np.float32
    ids = np.asarray(inputs["input_ids"], np.int32)          # [B, S]
    retw_raw = [np.asarray(inputs["ret_W"][l], np.float32) for l in range(L)]
    w1_raw = [np.asarray(inputs["ffn_W1"][l], np.float32) for l in range(L)]
    retb_raw = [np.asarray(inputs["ret_b"][l], np.float32) for l in range(L)]
    b1_raw = [np.asarray(inputs["ffn_b1"][l], np.float32) for l in range(L)]
    if ln_scaled:
        # fold LN scale/bias of the LN feeding each fused GEMM into W / bias
        for l in range(L):
            s_in = (np.asarray(inputs["emb_ln_s"], np.float32) if l == 0
                    else np.asarray(inputs["ln2_s"][l - 1], np.float32))
            b_in = (np.asarray(inputs["emb_ln_b"], np.float32) if l == 0
                    else np.asarray(inputs["ln2_b"][l - 1], np.float32))
            retb_raw[l] = retb_raw[l] + b_in @ retw_raw[l]
            retw_raw[l] = retw_raw[l] * s_in[:, None]
            s1 = np.asarray(inputs["ln1_s"][l], np.float32)
            b1_ = np.asarray(inputs["ln1_b"][l], np.float32)
            b1_raw[l] = b1_raw[l] + b1_ @ w1_raw[l]
            w1_raw[l] = w1_raw[l] * s1[:, None]
    csret = np.stack([_cols(w.sum(0), MH) for w in retw_raw])
    csw1 = np.stack([_cols(w.sum(0), MF) for w in w1_raw])
    retw = np.stack([_swz(w) for w in retw_raw]).astype(wdtype)
    w1 = np.stack([_swz(w) for w in w1_raw]).astype(wdtype)
    w2 = np.stack([_swz(np.asarray(inputs["ffn_W2"][l], np.float32))
                   for l in range(L)]).astype(wdtype)
    retb = np.stack([_cols(v, MH) for v in retb_raw])
    b1 = np.stack([_cols(v, MF) for v in b1_raw])
    b2 = np.stack([_cols(np.asarray(inputs["ffn_b2"][l], np.float32), MH)
                   for l in range(L)])
    lmw_full = np.asarray(inputs["lm_W"], np.float32)         # [H, V]
    pos_emb = np.asarray(inputs["pos_emb"], np.float32)       # [S, H]
    wemb = np.ascontiguousarray(np.asarray(inputs["word_emb"], np.float32))

    common = {
        "wemb": wemb, "retw": retw, "retb": retb,
        "w1": w1, "b1": b1, "w2": w2, "b2": b2,
        "csret": csret, "csw1": csw1,
    }
    if ln_scaled:
        slots = [( np.asarray(inputs["emb_ln_s"], np.float32),
                   np.asarray(inputs["emb_ln_b"], np.float32))]
        for l in range(L):
            slots.append((np.asarray(inputs["ln1_s"][l], np.float32),
                          np.asarray(inputs["ln1_b"][l], np.float32)))
            slots.append((np.asarray(inputs["ln2_s"][l], np.float32),
                          np.asarray(inputs["ln2_b"][l], np.float32)))
        slots.append((np.asarray(inputs["fin_ln_s"], np.float32),
                      np.asarray(inputs["fin_ln_b"], np.float32)))
        lns = np.stack([np.stack([_cols(s, MH), _cols(b, MH)]) for s, b in slots])
        common["lns"] = lns

    in_maps = []
    for c in range(NCORES):
        b = c // (NCORES // B)
        s0 = TM * (c % (NCORES // B))
        if s0 == 0:
            hids = ids[b, 0:HALO]
            hpos = np.arange(HALO)
        else:
            hids = ids[b, s0 - HALO:s0]
            hpos = np.arange(s0 - HALO, s0)
        cids = np.concatenate([hids, ids[b, s0:s0 + TM],
                               np.zeros(TPAD - T, np.int32)]).astype(np.int32)
        cpos = np.concatenate([hpos, np.arange(s0, s0 + TM),
                               np.zeros(TPAD - T, np.int64)])
        pos = pos_emb[cpos].reshape(3, 128, H)
        lmw_c = np.zeros((H, VSP), np.float32)
        lmw_c[:, :VS] = lmw_full[:, c * VS:(c + 1) * VS]
        m = dict(common)
        m["mask"] = np.full((128, 1), 0.0 if s0 == 0 else 1.0, np.float32)
        m["ids"] = cids.reshape(3, 128)
        m["pos"] = np.ascontiguousarray(pos)
        m["lmw"] = _swz(lmw_c).astype(wdtype)
        in_maps.append(m)
    return in_maps


def kernel(**inputs):
    gemm_bf16 = _os.environ.get("KERNEL_GEMM_DT", "bf16") == "bf16"
    trivial = all(
        np.allclose(np.asarray(inputs[k]), 1.0) for k in
        ("emb_ln_s", "ln1_s", "ln2_s", "fin_ln_s")
    ) and all(
        np.allclose(np.asarray(inputs[k]), 0.0) for k in
        ("emb_ln_b", "ln1_b", "ln2_b", "fin_ln_b")
    )
    ln_scaled = not trivial

    key = (gemm_bf16, ln_scaled)
    if key not in _compiled:
        _compiled[key] = _build(gemm_bf16, ln_scaled)
    nc = _compiled[key]

    in_maps = _prep_inputs(inputs, gemm_bf16, ln_scaled)
    trace = bool(_os.environ.get("KERNEL_TRACE"))
    if trace:
        _maybe_install_trace_hook()
    res = bass_utils.run_bass_kernel_spmd(
        nc, in_maps, core_ids=list(range(NCORES)), trace=trace)
    global LAST_EXEC_NS
    LAST_EXEC_NS = res.exec_time_ns

    logits = np.empty((TALL, V), np.float32)
    for c in range(NCORES):
        logits[:, c * VS:(c + 1) * VS] = res.results[c]["logits"][:VS, :].T
    return logits.reshape(B, S, V)
